# revision 48
# baseline (speedup 1.0000x reference)
"""Trainium2 Bass kernel for nn_BLCD_Loss (retrieval_knn).

Math: for l2-normalized rows, ||a-b||^2 = 2 - 2*a.b, so all pairwise
distances come from small Gram matmuls per core (bf16 inputs, fp32 PSUM
accumulate, 1 cycle/row on the PE). Per-core inputs are column-rolled so
every core's self-pair lands on the local diagonal; self-exclusion is a
core-uniform -BIG*I accumulated into the Gram via a third matmul whose
operands are generated on-device (affine_select). Column norms come from
a replicated ones-matmul (all 32 partitions get the column sums); local
row norms and the yi.yi_t cross dot come from [32,1] ones-matmuls over
elementwise-squared/multiplied local columns, landing per-partition with
no transpose. The normalized Grams are evicted from PSUM via otherwise-
idle Act-engine copies so the DVE multiply and max chain run at SBUF
speed. Top-16 selection is two rounds of DVE max8 + match_replace; the
masked sum of (dis - dis_t)^2 is one fused scalar_tensor_tensor
accumulate. All inputs ride one packed bf16 tensor split over two
SP-queue DMAs ordered so the critical column-norm chain lands first.
The v2 ASAP tile scheduler gives the best interleave.

Sharding: 256 anchor rows -> 32 rows on each of 8 cores; each core
returns [32,2] partial rows (e1, e2 terms); the host sums them.
"""

import numpy as np

N, D, K = 256, 256, 16
M_MARGIN, T_THRESH, EPS = 0.6, 0.0025, 1e-12
NCORES, RPC = 8, 32
BIG = 1.0e5

_CACHE = {}


def _build():
    import os
    os.environ["TILE_SCHEDULER"] = "asap"
    from concourse import bacc, mybir, tile
    import concourse.bass as bass

    dt = mybir.dt.float32
    bf = mybir.dt.bfloat16
    Alu = mybir.AluOpType
    Act = mybir.ActivationFunctionType

    nc = bacc.Bacc("TRN2", target_bir_lowering=False, debug=False)

    # pA cols (bf16): 0:32 yiLT0 | 32:64 yiLT1 | 64:96 yitT0 | 96:128 yitT1 |
    #                 128:384 yiT rows 0:128 (rolled) | 384:640 rows 128:256
    pA_d = nc.dram_tensor("pA", [128, 640], bf, kind="ExternalInput")
    out_d = nc.dram_tensor("out", [RPC, 2], dt, kind="ExternalOutput")

    with tile.TileContext(nc) as tc:
        with (
            tc.tile_pool(name="sb", bufs=1) as sb,
            tc.tile_pool(name="ps", bufs=1, space=bass.MemorySpace.PSUM) as ps,
        ):
            sbA = sb.tile([128, 640], bf)

            # ---- Pool: small constants + on-device identity operands
            cEPS = sb.tile([128, 1], dt)
            nc.gpsimd.memset(cEPS[:], EPS)
            cHALF = sb.tile([128, 1], dt)
            nc.gpsimd.memset(cHALF[:], 0.5)
            zz = sb.tile([RPC, N], bf)
            nc.gpsimd.memset(zz[:], 0.0)
            eyeN = sb.tile([RPC, N], bf)
            nc.gpsimd.affine_select(
                eyeN[:], zz[:], [[1, N]], Alu.not_equal, -BIG,
                base=0, channel_multiplier=-1)
            i32 = sb.tile([RPC, RPC], bf)
            nc.gpsimd.affine_select(
                i32[:], zz[:, 0:RPC], [[1, RPC]], Alu.not_equal, 1.0,
                base=0, channel_multiplier=-1)

            # ---- SP queue: the yiT block first, then the local columns
            nc.sync.dma_start(sbA[:, 128:640], pA_d[:, 128:640])
            nc.sync.dma_start(sbA[:, 0:128], pA_d[:, 0:128])

            yiT0 = sbA[:, 128:384]
            yiT1 = sbA[:, 384:640]
            yiLT0 = sbA[:, 0:32]
            yiLT1 = sbA[:, 32:64]
            yitT0 = sbA[:, 64:96]
            yitT1 = sbA[:, 96:128]

            # ---- Act: dummy sqrt first so the act table loads at t~0
            dummy = sb.tile([1, 1], dt)
            nc.scalar.activation(dummy[:], cEPS[0:1, :], Act.Sqrt,
                                 bias=cEPS[0:1, :], scale=1.0)

            # ---- DVE: ones + squared yiT halves (bf16, fast)
            ones = sb.tile([128, RPC], bf)
            nc.vector.memset(ones[:], 1.0)
            sqA0 = sb.tile([128, D], bf)
            nc.vector.tensor_tensor(sqA0[:], yiT0, yiT0, op=Alu.mult)
            sqA1 = sb.tile([128, D], bf)
            nc.vector.tensor_tensor(sqA1[:], yiT1, yiT1, op=Alu.mult)
            sqL = sb.tile([128, 64], bf)
            nc.vector.tensor_tensor(sqL[:], sbA[:, 0:64], sbA[:, 0:64],
                                    op=Alu.mult)
            sqLt = sb.tile([128, 64], bf)
            nc.vector.tensor_tensor(sqLt[:], sbA[:, 64:128], sbA[:, 64:128],
                                    op=Alu.mult)
            prodL = sb.tile([128, 64], bf)
            nc.vector.tensor_tensor(prodL[:], sbA[:, 0:64], sbA[:, 64:128],
                                    op=Alu.mult)

            # ---- PE: column sums (replicated), tiny local Grams, big Grams
            ps_s = ps.tile([RPC, N], dt)
            nc.tensor.matmul(ps_s[:], ones[:], sqA0[:], start=True, stop=False)
            nc.tensor.matmul(ps_s[:], ones[:], sqA1[:], start=False, stop=True)
            ps_n2 = ps.tile([RPC, 2], dt)
            nc.tensor.matmul(ps_n2[:, 0:1], sqL[:, 0:32], ones[:, 0:1],
                             start=True, stop=False, skip_group_check=True)
            nc.tensor.matmul(ps_n2[:, 0:1], sqL[:, 32:64], ones[:, 0:1],
                             start=False, stop=True, skip_group_check=True)
            nc.tensor.matmul(ps_n2[:, 1:2], sqLt[:, 0:32], ones[:, 0:1],
                             start=True, stop=False, skip_group_check=True)
            nc.tensor.matmul(ps_n2[:, 1:2], sqLt[:, 32:64], ones[:, 0:1],
                             start=False, stop=True, skip_group_check=True)
            ps_dx = ps.tile([RPC, 1], dt)
            nc.tensor.matmul(ps_dx[:], prodL[:, 0:32], ones[:, 0:1],
                             start=True, stop=False)
            nc.tensor.matmul(ps_dx[:], prodL[:, 32:64], ones[:, 0:1],
                             start=False, stop=True)
            ps_R = ps.tile([RPC, N], dt)
            nc.tensor.matmul(ps_R[:], yiLT0, yiT0, start=True, stop=False)
            nc.tensor.matmul(ps_R[:], yiLT1, yiT1, start=False, stop=False)
            nc.tensor.matmul(ps_R[:], i32[:], eyeN[:], start=False, stop=True)
            ps_Rt = ps.tile([RPC, N], dt)
            nc.tensor.matmul(ps_Rt[:], yitT0, yiT0, start=True, stop=False)
            nc.tensor.matmul(ps_Rt[:], yitT1, yiT1, start=False, stop=True)

            # ---- t_b = sqrt(colsum + eps); inv_b = 1/t_b
            t_b = sb.tile([RPC, N], dt)
            nc.scalar.activation(t_b[:], ps_s[:], Act.Sqrt,
                                 bias=cEPS[0:RPC, :], scale=1.0)
            t2 = sb.tile([RPC, 2], dt)
            nc.scalar.activation(t2[:], ps_n2[:], Act.Sqrt,
                                 bias=cEPS[0:RPC, :], scale=1.0)
            R_sb = sb.tile([RPC, N], dt)
            nc.scalar.activation(R_sb[:], ps_R[:], Act.Copy,
                                 bias=0.0, scale=1.0)
            Rt_sb = sb.tile([RPC, N], bf)
            nc.scalar.activation(Rt_sb[:], ps_Rt[:], Act.Copy,
                                 bias=0.0, scale=1.0)
            inv_b = sb.tile([RPC, N], dt)
            nc.vector.reciprocal(inv_b[:], t_b[:])
            inv2 = sb.tile([RPC, 2], dt)
            nc.vector.reciprocal(inv2[:], t2[:])
            sc2 = sb.tile([RPC, 2], dt)
            nc.gpsimd.tensor_scalar_mul(sc2[:], inv2[:], -0.5)

            # ---- normalized Grams (bf16 outputs for the fast top-k chain)
            work = sb.tile([RPC, N], dt)
            nc.vector.tensor_tensor(work[:], R_sb[:], inv_b[:], op=Alu.mult)
            # H1 on Pool so DVE can start the max chain immediately
            H1 = sb.tile([RPC, N], bf)
            nc.gpsimd.tensor_tensor(H1[:], Rt_sb[:], inv_b[:], op=Alu.mult)

            # ---- top-16 threshold per row (self sits at -BIG on the diag)
            m1 = sb.tile([RPC, 8], dt)
            nc.vector.max(out=m1[:], in_=work[:])
            w2 = sb.tile([RPC, N], dt)
            nc.vector.match_replace(
                out=w2[:], in_to_replace=m1[:], in_values=work[:],
                imm_value=-BIG)
            m2 = sb.tile([RPC, 8], dt)
            nc.vector.max(out=m2[:], in_=w2[:])

            # ---- distances (row scale folded into activation scale)
            dis = sb.tile([RPC, N], bf)
            nc.scalar.activation(dis[:], work[:], Act.Sqrt,
                                 bias=cHALF[0:RPC, :], scale=sc2[:, 0:1])
            dis_t = sb.tile([RPC, N], bf)
            nc.scalar.activation(dis_t[:], H1[:], Act.Sqrt,
                                 bias=cHALF[0:RPC, :], scale=sc2[:, 1:2])
            dis2 = sb.tile([RPC, 1], dt)
            nc.scalar.activation(dis2[:], m1[:, 0:1], Act.Sqrt,
                                 bias=cHALF[0:RPC, :], scale=sc2[:, 0:1])

            # ---- e2 = relu(dis_ii + (margin - dis2)) on the slack path
            u = sb.tile([RPC, 1], dt)
            nc.vector.tensor_tensor(u[:], ps_dx[:], inv2[:, 0:1], op=Alu.mult)
            dis_ii = sb.tile([RPC, 1], dt)
            nc.scalar.activation(dis_ii[:], u[:], Act.Sqrt,
                                 bias=cHALF[0:RPC, :], scale=sc2[:, 1:2])
            outsb = sb.tile([RPC, 2], dt)
            bias2 = sb.tile([RPC, 1], dt)
            nc.gpsimd.tensor_scalar(
                bias2[:], dis2[:], -1.0, M_MARGIN, op0=Alu.mult, op1=Alu.add)
            nc.scalar.activation(outsb[:, 1:2], dis_ii[:], Act.Relu,
                                 bias=bias2[:], scale=1.0)

            # ---- e1 = sum over neighbors of (dis - dis_t)^2, fused
            diff = sb.tile([RPC, N], bf)
            nc.vector.tensor_sub(diff[:], dis[:], dis_t[:])
            diffsq = sb.tile([RPC, N], bf)
            nc.vector.tensor_tensor(diffsq[:], diff[:], diff[:], op=Alu.mult)
            scr1 = sb.tile([RPC, N], bf)
            nc.vector.scalar_tensor_tensor(
                scr1[:], work[:], m2[:, 7:8], diffsq[:],
                op0=Alu.is_ge, op1=Alu.mult, accum_out=outsb[:, 0:1])

            nc.sync.dma_start(out_d[:, :], outsb[:])

    nc.compile()
    return nc


def _in_maps(yi, yi_t):
    import ml_dtypes
    bf16 = ml_dtypes.bfloat16
    yi = np.ascontiguousarray(np.asarray(yi, np.float32))
    yi_t = np.ascontiguousarray(np.asarray(yi_t, np.float32))
    yiT = yi.T
    maps = []
    for c in range(NCORES):
        r0 = c * RPC
        yiTp = np.roll(yiT, -r0, axis=1)
        pA = np.empty((128, 640), np.float32)
        pA[:, 0:32] = yi[r0:r0 + RPC, 0:128].T
        pA[:, 32:64] = yi[r0:r0 + RPC, 128:256].T
        pA[:, 64:96] = yi_t[r0:r0 + RPC, 0:128].T
        pA[:, 96:128] = yi_t[r0:r0 + RPC, 128:256].T
        pA[:, 128:384] = yiTp[0:128, :]
        pA[:, 384:640] = yiTp[128:256, :]
        maps.append({"pA": pA.astype(bf16)})
    return maps


def kernel(yi, yi_t):
    from concourse.bass_utils import run_bass_kernel_spmd

    if "nc" not in _CACHE:
        _CACHE["nc"] = _build()
    nc = _CACHE["nc"]
    res = run_bass_kernel_spmd(nc, _in_maps(yi, yi_t), list(range(NCORES)))
    total = np.float64(0.0)
    for c in range(NCORES):
        total += np.sum(res.results[c]["out"], dtype=np.float64)
    total -= np.float64(N * K * T_THRESH)
    return np.float32(total)
